# revision 27
# baseline (speedup 1.0000x reference)
"""Trainium2 Bass kernel for nn_CosmicBaseModel (dense transformer block).

Computation (per batch element b):
    E = X @ W_enc + b_enc            [S, D]
    S_mat = E @ E^T                  [S, S]   (no 1/sqrt(d) scale, no mask)
    P = softmax(S_mat, axis=-1)
    A = P @ E
    Y = A @ W_dec + b_dec            [S, H]

Sharding: data-parallel over batch, one batch element per NeuronCore (B=8,
8 cores).

Per-core structure (H=256 < D=512, so the score matrix has rank <= H+1 and
everything routes through host-precomputed HxH products):
    G  = W_enc @ W_enc^T, w_b = W_enc @ b_enc          (host)
    W' = W_enc @ W_dec,   b' = b_enc @ W_dec + b_dec   (host)
    S_mat = X G X^T + u 1^T + 1 u^T + |b|^2,  u = X w_b
    V  = [X W' + 1 b'^T | 1]                            [S, H+2]
    Y  = (P~ @ V)[:, :H] / (P~ @ V)[:, H]
with P~ = exp(S - shift). The shift is the score diagonal approximation
xGx_ss (colsum of F^T * X^T where F^T = G X^T), which dominates every
off-diagonal for random high-dim features; softmax shift-invariance is
exact here because the denominator comes from the same P~ via the ones
column of V, so any per-row shift error cancels identically.

All matmuls use float32r (fp32 operands at 1 cycle/row on the PE for
moving free dims >= 256; f32r moving widths must be even). P~ is
materialized in bf16 to halve the PE transpose cost; the values are
exp(<=~0) where off-diagonals underflow, and the row normalization
cancels rounding of the dominant diagonal entry.
"""

import sys

if "/opt/trn_rl_repo" not in sys.path:
    sys.path.insert(0, "/opt/trn_rl_repo")

import numpy as np

B, S, H, D = 8, 2048, 256, 512
P = 128
NS = S // P    # 16 s-tiles
NH = H // P    # 2 h partition blocks
CH = 512       # free-dim chunk for scores / psum bank
NCH = S // CH  # 4 chunks
HP = H + 2     # V width: decode output | softmax denom | pad

_CACHE = {}


def _build_nc(repeat=1):
    import contextlib

    import concourse.bacc as bacc
    import concourse.mybir as mybir
    import concourse.tile as tile

    f32 = mybir.dt.float32
    f32r = mybir.dt.float32r
    bf16 = mybir.dt.bfloat16
    Act = mybir.ActivationFunctionType

    nc = bacc.Bacc("TRN2", target_bir_lowering=False, debug=False)

    xT_d = nc.dram_tensor("xT", [H, S], f32r, kind="ExternalInput")
    g_d = nc.dram_tensor("g_mat", [H, H], f32r, kind="ExternalInput")
    wb_d = nc.dram_tensor("w_b2", [P, 2 * NH], f32r, kind="ExternalInput")
    wp_d = nc.dram_tensor("w_prime", [H, HP], f32r, kind="ExternalInput")
    bp_d = nc.dram_tensor("b_prime2", [2, HP], f32r, kind="ExternalInput")
    o2_d = nc.dram_tensor("ones2", [2, P], f32r, kind="ExternalInput")
    oc_d = nc.dram_tensor("ones_col", [P, 2], f32r, kind="ExternalInput")
    ib_d = nc.dram_tensor("ident_bf16", [P, P], bf16, kind="ExternalInput")
    y_d = nc.dram_tensor("y", [S, H], f32, kind="ExternalOutput")

    with tile.TileContext(nc) as tc:
        with (
            tc.tile_pool(name="const", bufs=1) as cpool,
            tc.tile_pool(name="persist", bufs=1) as ppool,
            tc.tile_pool(name="p_sb", bufs=2) as p_pool,
            tc.tile_pool(name="pT_sb", bufs=2) as pT_pool,
            tc.tile_pool(name="stats", bufs=3) as st_pool,
            tc.tile_pool(name="ysb", bufs=3) as y_pool,
            tc.tile_pool(name="psA", bufs=3, space="PSUM") as psA,  # [P,512] f32
            tc.tile_pool(name="psT", bufs=2, space="PSUM") as psT,  # [P,512] bf16
            tc.tile_pool(name="psB", bufs=2, space="PSUM") as psB,  # [P,HP] f32
            tc.tile_pool(name="psM", bufs=1, space="PSUM") as psM,  # [P,2NS]+[2,CH]
            tc.For_i(
                0, repeat, 1,
                hint_engines=(
                    mybir.EngineType.PE,
                    mybir.EngineType.Activation,
                    mybir.EngineType.DVE,
                    mybir.EngineType.Pool,
                    mybir.EngineType.SP,
                ),
            ) if repeat > 1 else contextlib.nullcontext(),
        ):
            # ---- constants / weights ----
            ident_b = cpool.tile([P, P], bf16, tag="ident_b")
            nc.sync.dma_start(ident_b[:], ib_d[:])
            ones2 = cpool.tile([2, P], f32r, tag="ones2")
            nc.sync.dma_start(ones2[:], o2_d[:])
            ones_col = cpool.tile([P, 2], f32r, tag="ones_col")
            nc.sync.dma_start(ones_col[:], oc_d[:])
            g_sb = [cpool.tile([P, H], f32r, tag=f"g{k}", name=f"g{k}")
                    for k in range(NH)]
            for k in range(NH):
                nc.sync.dma_start(g_sb[k][:], g_d[k * P:(k + 1) * P, :])
            wb_sb = cpool.tile([P, 2 * NH], f32r, tag="wb")
            nc.sync.dma_start(wb_sb[:], wb_d[:])
            wp_sb = [cpool.tile([P, HP], f32r, tag=f"wp{k}", name=f"wp{k}")
                     for k in range(NH)]
            for k in range(NH):
                nc.sync.dma_start(wp_sb[k][:], wp_d[k * P:(k + 1) * P, :])
            bp_sb = cpool.tile([2, HP], f32r, tag="bp")
            nc.sync.dma_start(bp_sb[:], bp_d[:])

            # xT in 2x4 chunks so downstream work starts before the full load
            xT_sb = [
                [ppool.tile([P, CH], f32r, tag=f"xT{k}_{n}", name=f"xT{k}_{n}")
                 for n in range(NCH)]
                for k in range(NH)
            ]
            for k in range(NH):
                for n in range(NCH):
                    nc.sync.dma_start(
                        xT_sb[k][n][:],
                        xT_d[k * P:(k + 1) * P, n * CH:(n + 1) * CH],
                    )

            # ---- F^T = G @ X^T  -> [H, S] (G symmetric) ----
            fT = [ppool.tile([P, S], f32r, tag=f"fT{m}", name=f"fT{m}")
                  for m in range(NH)]
            for n in range(NCH):
                for m in range(NH):
                    ps = psA.tile([P, CH], f32, tag="psA")
                    for k in range(NH):
                        nc.tensor.matmul(
                            ps[:],
                            lhsT=g_sb[k][:, m * P:(m + 1) * P],
                            rhs=xT_sb[k][n][:],
                            start=(k == 0),
                            stop=(k == NH - 1),
                        )
                    nc.vector.tensor_copy(fT[m][:, n * CH:(n + 1) * CH], ps[:])

            # ---- u_t = x_t . w_b, then e^u: the rank-1 score column
            # term u 1^T moves out of the softmax argument and into V as a
            # per-row scale e^{u_t} (softmax column reweighting), which is
            # applied for free on the V PSUM->SBUF copy. ----
            pu = psM.tile([P, 2 * NS], f32, tag="psm", name="pu")
            for t in range(NS):
                for k in range(NH):
                    nc.tensor.matmul(
                        pu[:, 2 * t:2 * t + 2],
                        lhsT=xT_sb[k][t // 4][:, (t % 4) * P:(t % 4 + 1) * P],
                        rhs=wb_sb[:, 2 * k:2 * k + 2],
                        start=(k == 0),
                        stop=(k == NH - 1),
                    )
            eu_all = cpool.tile([P, 2 * NS], f32, tag="eu_all")
            nc.scalar.activation(eu_all[:], pu[:], Act.Exp)

            # ---- V[t] = [X W' + 1 b'^T | 1 | 0]  -> [S, HP], t-blocked ----
            V = [ppool.tile([P, HP], f32r, tag=f"V{t}", name=f"Vt{t}")
                 for t in range(NS)]
            for t in range(NS):
                pv = psB.tile([P, HP], f32, tag="psB")
                for k in range(NH):
                    nc.tensor.matmul(
                        pv[:],
                        lhsT=xT_sb[k][t // 4][:, (t % 4) * P:(t % 4 + 1) * P],
                        rhs=wp_sb[k][:],
                        start=(k == 0),
                        stop=False,
                    )
                nc.tensor.matmul(
                    pv[:], lhsT=ones2[:], rhs=bp_sb[:],
                    start=False, stop=True,
                )
                nc.scalar.activation(
                    V[t][:], pv[:], Act.Copy, scale=eu_all[:, 2 * t:2 * t + 1],
                )

            # ---- softmax shift: m_s = xGx_ss = colsum(F^T * X^T)[s],
            # via (F^T*X^T) contracted with a ones column ----
            prod = [ppool.tile([P, S], f32r, tag=f"prod{m}", name=f"prod{m}")
                    for m in range(NH)]
            for m in range(NH):
                for n in range(NCH):
                    nc.vector.tensor_mul(
                        prod[m][:, n * CH:(n + 1) * CH],
                        fT[m][:, n * CH:(n + 1) * CH],
                        xT_sb[m][n][:],
                    )
            msq = psM.tile([P, 2 * NS], f32, tag="psm", name="msq")
            mnegs = []
            for i in range(NS):
                for k in range(NH):
                    nc.tensor.matmul(
                        msq[:, 2 * i:2 * i + 2],
                        lhsT=prod[k][:, i * P:(i + 1) * P],
                        rhs=ones_col[:],
                        start=(k == 0),
                        stop=(k == NH - 1),
                    )
                # per-tile negated copy so exp(i) only waits on its own
                # msq group, not the whole phase
                mn = cpool.tile([P, 2], f32, tag=f"mneg{i}", name=f"mneg{i}")
                mnegs.append(mn)
                nc.scalar.activation(mn[:], msq[:, 2 * i:2 * i + 2],
                                     Act.Copy, scale=-1.0)

            # ---- attention + fused decode, software-pipelined over s-tiles:
            # stage A(i): score matmuls + exp; stage B(i): transposes, PV,
            # normalize, store. Emitted A(0), A(1), B(0), A(2), B(1), ...
            # so the PE never waits on the exp of the tile it transposes. ----
            def stage_a(i):
                p_sb = p_pool.tile([P, S], bf16, tag="p", name=f"p{i}")
                for n in range(NCH):
                    spn = psA.tile([P, CH], f32, tag="psA")
                    for k in range(NH):
                        nc.tensor.matmul(
                            spn[:],
                            lhsT=fT[k][:, i * P:(i + 1) * P],
                            rhs=xT_sb[k][n][:],
                            start=(k == 0),
                            stop=(k == NH - 1),
                        )
                    nc.scalar.activation(
                        p_sb[:, n * CH:(n + 1) * CH], spn[:],
                        Act.Exp, bias=mnegs[i][:, 0:1], scale=1.0,
                    )
                return (p_sb,)

            def stage_b(i, p_sb):
                pT = []
                for g in range(NCH):
                    tp = psT.tile([P, CH], bf16, tag="psT")
                    for jj in range(4):
                        j = g * 4 + jj
                        nc.tensor.transpose(
                            tp[:, jj * P:(jj + 1) * P],
                            p_sb[:, j * P:(j + 1) * P],
                            ident_b[:],
                        )
                    # cast back to f32r on the PSUM->SBUF copy: the PV
                    # matmul may not mix bf16 and f32r operands
                    pTg = pT_pool.tile([P, CH], f32r, tag=f"pT{g}",
                                       name=f"pT{g}_{i}")
                    pT.append(pTg)
                    if g % 4 == 3:
                        nc.scalar.copy(pTg[:], tp[:])
                    else:
                        nc.vector.tensor_copy(pTg[:], tp[:])

                pvp = psB.tile([P, HP], f32, tag="psB")
                for j in range(NS):
                    nc.tensor.matmul(
                        pvp[:],
                        lhsT=pT[j // 4][:, (j % 4) * P:(j % 4 + 1) * P],
                        rhs=V[j][:],
                        start=(j == 0),
                        stop=(j == NS - 1),
                    )
                # y_i = PV[:, :H] / PV[:, H]  (decode bias already inside V)
                r = st_pool.tile([P, 1], f32, tag="r", name=f"r{i}")
                nc.vector.reciprocal(r[:], pvp[:, H:H + 1])
                y_sb = y_pool.tile([P, H], f32, tag="y", name=f"y{i}")
                nc.scalar.activation(y_sb[:], pvp[:, 0:H], Act.Copy, scale=r[:])
                nc.sync.dma_start(y_d[i * P:(i + 1) * P, :], y_sb[:])

            prev = (0, *stage_a(0))
            for i in range(1, NS):
                cur = (i, *stage_a(i))
                stage_b(*prev)
                prev = cur
            stage_b(*prev)

    nc.compile()
    return nc


def _get_nc():
    if "nc" not in _CACHE:
        _CACHE["nc"] = _build_nc()
    return _CACHE["nc"]


def _make_in_maps(cosmic_input, W_enc, b_enc, W_dec, b_dec):
    import ml_dtypes

    x = np.asarray(cosmic_input, dtype=np.float32)
    We = np.asarray(W_enc, dtype=np.float64)
    Wd = np.asarray(W_dec, dtype=np.float64)
    be = np.asarray(b_enc, dtype=np.float64)
    bd = np.asarray(b_dec, dtype=np.float64)

    G = (We @ We.T).astype(np.float32)                    # [H, H]
    w_b = (We @ be).astype(np.float32)                    # [H]
    Wp = (We @ Wd).astype(np.float32)                     # [H, H]
    bp = (be @ Wd + bd).astype(np.float32)                # [H]

    # wb2[p, 2k+c]: column 2k holds w_b block k, odd columns zero
    wb2 = np.zeros((P, 2 * NH), np.float32)
    for k in range(NH):
        wb2[:, 2 * k] = w_b[k * P:(k + 1) * P]
    wp_pad = np.zeros((H, HP), np.float32)
    wp_pad[:, 0:H] = Wp
    bp2 = np.zeros((2, HP), np.float32)
    bp2[0, 0:H] = bp
    bp2[0, H] = 1.0
    ones2 = np.zeros((2, P), np.float32)
    ones2[0, :] = 1.0

    shared = {
        "g_mat": np.ascontiguousarray(G),
        "w_b2": wb2,
        "w_prime": wp_pad,
        "b_prime2": bp2,
        "ones2": ones2,
        "ones_col": np.ones((P, 2), dtype=np.float32),
        "ident_bf16": np.eye(P, dtype=ml_dtypes.bfloat16),
    }
    return [
        {"xT": np.ascontiguousarray(x[b].T), **shared} for b in range(B)
    ]


def kernel(cosmic_input, W_enc, b_enc, W_dec, b_dec):
    from concourse import bass_utils

    nc = _get_nc()
    in_maps = _make_in_maps(cosmic_input, W_enc, b_enc, W_dec, b_dec)
    res = bass_utils.run_bass_kernel_spmd(nc, in_maps, core_ids=list(range(B)))
    out = np.stack([res.results[b]["y"] for b in range(B)], axis=0)
    return out.astype(np.float32)
